# revision 1
# baseline (speedup 1.0000x reference)
"""HAN layer (3-metapath GraphConv + semantic attention) on 8 trn2 NeuronCores.

Strategy (per sharding hint): shard destination nodes across the 8 cores
(6250 rows each), partition each metapath's edge list by destination shard
on the host, and sort/pad it into fixed-size 128-edge chunks per 128-dst
output tile.  Each core gathers source rows of h with indirect DMA (h is
replicated in every core's DRAM), aggregates with a selection-matrix matmul
into PSUM (z stored d-major in SBUF), computes its shard's partial semantic
attention score, all-reduces the tiny [1,3] score vector across cores,
applies softmax on-device, and writes its 6250-row slice of the output.
"""

import numpy as np

import concourse.bass as bass
import concourse.bacc as bacc
import concourse.mybir as mybir
import concourse.tile as tile
from concourse.bass_utils import run_bass_kernel_spmd

P = 128
N = 50000
D = 128
M = 3
E = 1_600_000
NCORES = 8
NSH = N // NCORES          # 6250 dst rows per core
NTILES = (NSH + P - 1) // P  # 49 output tiles (last has 106 real rows)

TRACE = False
LAST_RESULTS = None

_PROGRAM_CACHE = {}


def _preprocess(edges):
    """Host-side: per-core, per-metapath padded chunk streams.

    Returns (offs_all, dstrel_all, wgt_all, C) where each array is
    [NCORES][P, M*NTILES*C]; column (m*NTILES + t)*C + c, lane p holds edge
    (tile t, chunk c, position p) of metapath m on that core.
    """
    per_core = [[] for _ in range(NCORES)]  # per core: list over m of dicts
    counts = []
    for m in range(M):
        src = np.asarray(edges[m, 0])
        dst = np.asarray(edges[m, 1])
        out_deg = np.bincount(src, minlength=N).astype(np.float32)
        in_deg = np.bincount(dst, minlength=N).astype(np.float32)
        ns = 1.0 / np.sqrt(np.maximum(out_deg, 1.0))
        nd = 1.0 / np.sqrt(np.maximum(in_deg, 1.0))
        w_e = (ns[src] * nd[dst]).astype(np.float32)

        order = np.argsort(dst, kind="stable")
        src_s, dst_s, w_s = src[order], dst[order], w_e[order]
        shard_bounds = np.searchsorted(dst_s, NSH * np.arange(NCORES + 1))
        for core in range(NCORES):
            lo, hi = shard_bounds[core], shard_bounds[core + 1]
            sc, dc, wc = src_s[lo:hi], dst_s[lo:hi] - core * NSH, w_s[lo:hi]
            tile_id = dc >> 7  # dst_local // 128
            tile_bounds = np.searchsorted(tile_id, np.arange(NTILES + 1))
            cnts = np.diff(tile_bounds)
            counts.append(cnts)
            per_core[core].append((sc, dc, wc, tile_bounds))

    C = int(np.ceil(max(int(c.max()) for c in counts) / P))
    ncol = M * NTILES * C

    offs_all, dstrel_all, wgt_all = [], [], []
    for core in range(NCORES):
        offs = np.zeros((NTILES * M, C * P), np.int32)
        drel = np.zeros((NTILES * M, C * P), np.float32)
        wgt = np.zeros((NTILES * M, C * P), np.float32)
        for m in range(M):
            sc, dc, wc, tb = per_core[core][m]
            for t in range(NTILES):
                lo, hi = tb[t], tb[t + 1]
                n = hi - lo
                row = m * NTILES + t
                offs[row, :n] = sc[lo:hi]
                drel[row, :n] = (dc[lo:hi] - t * P).astype(np.float32)
                wgt[row, :n] = wc[lo:hi]
        # [M*NTILES, C*P] -> [M*NTILES, C, P] -> [P, M*NTILES*C]
        offs_all.append(
            np.ascontiguousarray(
                offs.reshape(ncol, P).T
            )
        )
        dstrel_all.append(np.ascontiguousarray(drel.reshape(ncol, P).T))
        wgt_all.append(np.ascontiguousarray(wgt.reshape(ncol, P).T))
    return offs_all, dstrel_all, wgt_all, C


def _build_program(C):
    if C in _PROGRAM_CACHE:
        return _PROGRAM_CACHE[C]
    f32 = mybir.dt.float32
    ncol = M * NTILES * C

    nc = bacc.Bacc(
        None, target_bir_lowering=False, num_devices=NCORES, num_swdge_queues=4
    )
    h_d = nc.dram_tensor("h", [N, D], f32, kind="ExternalInput")
    offs_d = nc.dram_tensor("offs", [P, ncol], mybir.dt.int32, kind="ExternalInput")
    drel_d = nc.dram_tensor("drel", [P, ncol], f32, kind="ExternalInput")
    wgt_d = nc.dram_tensor("wgt", [P, ncol], f32, kind="ExternalInput")
    iota_d = nc.dram_tensor("iota", [P, P], f32, kind="ExternalInput")
    ident_d = nc.dram_tensor("ident", [P, P], f32, kind="ExternalInput")
    w1_d = nc.dram_tensor("w1", [D, D], f32, kind="ExternalInput")
    w2_d = nc.dram_tensor("w2", [D, 1], f32, kind="ExternalInput")
    b1_d = nc.dram_tensor("b1", [D, 1], f32, kind="ExternalInput")
    out_d = nc.dram_tensor("out", [NSH, D], f32, kind="ExternalOutput")

    with tile.TileContext(nc) as tc:
        with (
            tc.tile_pool(name="consts", bufs=1) as cpool,
            tc.tile_pool(name="zpool", bufs=1) as zpool,
        ):
          with (
            tc.tile_pool(name="meta", bufs=3) as mpool,
            tc.tile_pool(name="gather", bufs=3) as gpool,
            tc.tile_pool(name="sel", bufs=4) as spool,
            tc.tile_pool(name="psum_z", bufs=2, space="PSUM") as pz,
          ):
            iota_sb = cpool.tile([P, P], f32)
            nc.sync.dma_start(out=iota_sb[:], in_=iota_d[:])
            ident_sb = cpool.tile([P, P], f32)
            nc.sync.dma_start(out=ident_sb[:], in_=ident_d[:])
            w1_sb = cpool.tile([P, P], f32)
            nc.sync.dma_start(out=w1_sb[:], in_=w1_d[:])
            w2_sb = cpool.tile([P, 1], f32)
            nc.sync.dma_start(out=w2_sb[:], in_=w2_d[:])
            b1_sb = cpool.tile([P, 1], f32)
            nc.sync.dma_start(out=b1_sb[:], in_=b1_d[:])

            # z stored d-major: z^T[m] tile t lives at columns (m*NTILES+t)*P
            z_sb = zpool.tile([P, M * NTILES * P], f32)

            # ---- sparse aggregation ----
            for m in range(M):
                for t in range(NTILES):
                    col0 = (m * NTILES + t) * C
                    o_sb = mpool.tile([P, C], mybir.dt.int32, tag="offs")
                    nc.sync.dma_start(
                        out=o_sb[:], in_=offs_d[:, col0 : col0 + C]
                    )
                    dr_sb = mpool.tile([P, C], f32, tag="drel")
                    nc.sync.dma_start(
                        out=dr_sb[:], in_=drel_d[:, col0 : col0 + C]
                    )
                    wg_sb = mpool.tile([P, C], f32, tag="wgt")
                    nc.sync.dma_start(
                        out=wg_sb[:], in_=wgt_d[:, col0 : col0 + C]
                    )
                    g_sb = gpool.tile([P, C * P], f32, tag="g")
                    for c in range(C):
                        nc.gpsimd.indirect_dma_start(
                            out=g_sb[:, c * P : (c + 1) * P],
                            out_offset=None,
                            in_=h_d[:],
                            in_offset=bass.IndirectOffsetOnAxis(
                                ap=o_sb[:, c : c + 1], axis=0
                            ),
                        )
                    psum_zt = pz.tile([P, P], f32, space="PSUM", tag="zt")
                    for c in range(C):
                        s_sb = spool.tile([P, P], f32, tag="s")
                        nc.vector.tensor_scalar(
                            out=s_sb[:],
                            in0=iota_sb[:],
                            scalar1=dr_sb[:, c : c + 1],
                            scalar2=wg_sb[:, c : c + 1],
                            op0=mybir.AluOpType.is_equal,
                            op1=mybir.AluOpType.mult,
                        )
                        nc.tensor.matmul(
                            out=psum_zt[:],
                            lhsT=g_sb[:, c * P : (c + 1) * P],
                            rhs=s_sb[:],
                            start=(c == 0),
                            stop=(c == C - 1),
                        )
                    nc.vector.tensor_copy(
                        out=z_sb[:, (m * NTILES + t) * P : (m * NTILES + t + 1) * P],
                        in_=psum_zt[:],
                    )

          # ---- semantic attention epilogue ----
          with (
            tc.tile_pool(name="epi_psum", bufs=2, space="PSUM") as ep,
            tc.tile_pool(name="score_psum", bufs=1, space="PSUM") as sp,
            tc.tile_pool(name="out_psum", bufs=2, space="PSUM") as op_,
            tc.tile_pool(name="epi_sbuf", bufs=2) as es,
            tc.tile_pool(name="small", bufs=1) as sm,
          ):
            psum_s = sp.tile([1, M * P], f32, space="PSUM")
            for m in range(M):
                for t in range(NTILES):
                    zt = z_sb[:, (m * NTILES + t) * P : (m * NTILES + t + 1) * P]
                    psum_y = ep.tile([P, P], f32, space="PSUM", tag="y")
                    nc.tensor.matmul(
                        out=psum_y[:], lhsT=w1_sb[:], rhs=zt, start=True, stop=True
                    )
                    tanh_sb = es.tile([P, P], f32, tag="tanh")
                    nc.scalar.activation(
                        out=tanh_sb[:],
                        in_=psum_y[:],
                        func=mybir.ActivationFunctionType.Tanh,
                        bias=b1_sb[:, :1],
                    )
                    nc.tensor.matmul(
                        out=psum_s[:, m * P : (m + 1) * P],
                        lhsT=w2_sb[:],
                        rhs=tanh_sb[:],
                        start=(t == 0),
                        stop=(t == NTILES - 1),
                    )
            wrow = sm.tile([1, M], f32)
            for m in range(M):
                nc.vector.reduce_sum(
                    out=wrow[:, m : m + 1],
                    in_=psum_s[:, m * P : (m + 1) * P],
                    axis=mybir.AxisListType.X,
                )
            with tc.tile_pool(name="ccdram", bufs=1, space="DRAM") as ccp:
                cc_in_t = ccp.tile([1, M], f32)
                cc_out_t = ccp.tile([1, M], f32, addr_space="Shared")
                nc.gpsimd.dma_start(cc_in_t[:], wrow[:])
                nc.gpsimd.collective_compute(
                    "AllReduce",
                    mybir.AluOpType.add,
                    replica_groups=[list(range(NCORES))],
                    ins=[cc_in_t.opt()],
                    outs=[cc_out_t.opt()],
                )
                w_bc = sm.tile([P, M], f32)
                nc.sync.dma_start(
                    out=w_bc[:], in_=cc_out_t[0:1, :].to_broadcast([P, M])
                )
            # softmax over the M columns (identical on every partition)
            nc.vector.tensor_scalar(
                out=w_bc[:],
                in0=w_bc[:],
                scalar1=1.0 / N,
                scalar2=None,
                op0=mybir.AluOpType.mult,
            )
            negmax = sm.tile([P, 1], f32)
            nc.vector.tensor_reduce(
                out=negmax[:],
                in_=w_bc[:],
                axis=mybir.AxisListType.X,
                op=mybir.AluOpType.max,
                negate=True,
            )
            e_bc = sm.tile([P, M], f32)
            nc.scalar.activation(
                out=e_bc[:],
                in_=w_bc[:],
                func=mybir.ActivationFunctionType.Exp,
                bias=negmax[:, :1],
            )
            esum = sm.tile([P, 1], f32)
            nc.vector.reduce_sum(
                out=esum[:], in_=e_bc[:], axis=mybir.AxisListType.X
            )
            rsum = sm.tile([P, 1], f32)
            nc.vector.reciprocal(out=rsum[:], in_=esum[:])
            beta = sm.tile([P, M], f32)
            nc.vector.tensor_scalar(
                out=beta[:],
                in0=e_bc[:],
                scalar1=rsum[:, :1],
                scalar2=None,
                op0=mybir.AluOpType.mult,
            )
            ibeta = sm.tile([P, M * P], f32)
            for m in range(M):
                nc.vector.tensor_scalar(
                    out=ibeta[:, m * P : (m + 1) * P],
                    in0=ident_sb[:],
                    scalar1=beta[:, m : m + 1],
                    scalar2=None,
                    op0=mybir.AluOpType.mult,
                )
            # ---- final combine: out tile = sum_m z_m^T(tile)^T @ (I * beta_m) ----
            for t in range(NTILES):
                psum_o = op_.tile([P, P], f32, space="PSUM", tag="o")
                for m in range(M):
                    nc.tensor.matmul(
                        out=psum_o[:],
                        lhsT=z_sb[:, (m * NTILES + t) * P : (m * NTILES + t + 1) * P],
                        rhs=ibeta[:, m * P : (m + 1) * P],
                        start=(m == 0),
                        stop=(m == M - 1),
                    )
                rows = min(P, NSH - t * P)
                o_sb = es.tile([P, P], f32, tag="out")
                nc.vector.tensor_copy(out=o_sb[:], in_=psum_o[:])
                nc.sync.dma_start(
                    out=out_d[t * P : t * P + rows, :], in_=o_sb[:rows, :]
                )
    nc.finalize()
    _PROGRAM_CACHE[C] = nc
    return nc


def kernel(h, edges, W1, b1, W2):
    global LAST_RESULTS
    h = np.ascontiguousarray(np.asarray(h, dtype=np.float32))
    edges = np.asarray(edges)
    offs_all, dstrel_all, wgt_all, C = _preprocess(edges)
    nc = _build_program(C)

    iota = np.tile(np.arange(P, dtype=np.float32), (P, 1))
    ident = np.eye(P, dtype=np.float32)
    w1 = np.ascontiguousarray(np.asarray(W1, dtype=np.float32))
    w2 = np.ascontiguousarray(np.asarray(W2, dtype=np.float32).reshape(D, 1))
    b1c = np.ascontiguousarray(np.asarray(b1, dtype=np.float32).reshape(D, 1))

    in_maps = []
    for core in range(NCORES):
        in_maps.append(
            {
                "h": h,
                "offs": offs_all[core],
                "drel": dstrel_all[core],
                "wgt": wgt_all[core],
                "iota": iota,
                "ident": ident,
                "w1": w1,
                "w2": w2,
                "b1": b1c,
            }
        )
    res = run_bass_kernel_spmd(
        nc, in_maps, core_ids=list(range(NCORES)), trace=TRACE
    )
    LAST_RESULTS = res
    out = np.concatenate([res.results[c]["out"] for c in range(NCORES)], axis=0)
    return out



# revision 10
# speedup vs baseline: 1.3436x; 1.3436x over previous
"""HAN layer (3-metapath GraphConv + semantic attention) on 8 trn2 NeuronCores.

Strategy: shard destination nodes across the 8 cores (6250 rows each) and
partition each metapath's edge list by destination shard on the host.  Edges
are sorted by destination and cut into 128-edge chunks per 128-dst output
tile; chunk counts are padded to the cross-core maximum so one SPMD program
serves all cores.

The per-edge source-row gather uses the SWDGE `dma_gather` instruction
(one descriptor per index, max 1024 indices per instruction = the 16KiB
descriptor-ring limit).  Its indices are int16, so h is pre-cast to bf16 on
the host and split into two 25000-row halves; each tile's edges are split
into lo/hi chunk runs accordingly.

Because the GraphConv edge weight is separable (w_e = ns[src]*nd[dst]) it is
folded into the per-chunk selection matrix S[e, dst] = (dst_rel[e] == iota)
* w_e, built with one 4x-mode tensor_scalar (is_equal + mult) on DVE.  The
aggregation is one bf16 matmul per chunk accumulating z^T = g^T @ S
(d-major) in PSUM (per-tile accumulation across the lo and hi chunk runs).

The semantic-attention score path runs fused per 8-tile group: y^T = W1^T @
z^T (N=512 matmuls), tanh+bias on the Scalar engine with accum_out giving
per-partition column sums.  The tiny remainder (W2 dot, softmax over the 3
metapaths, beta-weighted combine of z, final transpose) runs on the host,
which removes the all-reduce and the device-side combine entirely.
"""

import numpy as np
import ml_dtypes

import concourse.bass as bass
import concourse.bacc as bacc
import concourse.mybir as mybir
import concourse.tile as tile
from concourse.bass_utils import run_bass_kernel_spmd

BF16 = ml_dtypes.bfloat16

P = 128
N = 50000
HALF = 25000
D = 128
M = 3
NCORES = 8
NSH = N // NCORES            # 6250 dst rows per core
NTILES = (NSH + P - 1) // P  # 49 output tiles (last has 106 real rows)
G = 4                        # dst tiles per group
GROUPS = [(t0, min(G, NTILES - t0)) for t0 in range(0, NTILES, G)]  # 13 groups
NGRP = len(GROUPS)
MAXIDX = 1024                # dma_gather descriptor-ring limit

TRACE = False
LAST_RESULTS = None

_PROGRAM_CACHE = {}


def _preprocess(edges):
    """Host-side prep.

    Chunk stream order (shared by idx / drw / device program):
      for m, for (t0,gt) group: [lo chunks of t0..t0+gt-1] + [hi chunks ...]

    Returns:
      idx_all[core]:  int16 [P, total_lanes//16]  (16-partition wrapped, x8)
      drw_all[core]:  f32  [P, 2*total_chunks]    per-group [dr cols | w cols]
      meta: tuple describing the uniform program structure
    """
    per_core = [[] for _ in range(NCORES)]
    for m in range(M):
        src = np.asarray(edges[m, 0])
        dst = np.asarray(edges[m, 1])
        out_deg = np.bincount(src, minlength=N).astype(np.float64)
        in_deg = np.bincount(dst, minlength=N).astype(np.float64)
        ns = 1.0 / np.sqrt(np.maximum(out_deg, 1.0))
        nd = 1.0 / np.sqrt(np.maximum(in_deg, 1.0))
        w_e = (ns[src] * nd[dst]).astype(np.float32)

        order = np.argsort(dst, kind="stable")
        src_s, dst_s, w_s = src[order], dst[order], w_e[order]
        shard_bounds = np.searchsorted(dst_s, NSH * np.arange(NCORES + 1))
        for core in range(NCORES):
            lo, hi = shard_bounds[core], shard_bounds[core + 1]
            sc, dc, wc = src_s[lo:hi], dst_s[lo:hi] - core * NSH, w_s[lo:hi]
            tile_id = dc >> 7
            tb = np.searchsorted(tile_id, np.arange(NTILES + 1))
            # per tile: split by src half, store (idx, drel, w) arrays
            tiles = []
            for t in range(NTILES):
                a, b = tb[t], tb[t + 1]
                s_t, d_t, w_t = sc[a:b], dc[a:b] - t * P, wc[a:b]
                is_lo = s_t < HALF
                tiles.append(
                    (
                        (s_t[is_lo], d_t[is_lo], w_t[is_lo]),
                        (s_t[~is_lo] - HALF, d_t[~is_lo], w_t[~is_lo]),
                    )
                )
            per_core[core].append(tiles)

    # cross-core max chunk counts -> uniform program
    C_half = np.zeros((2, M, NTILES), np.int64)
    for m in range(M):
        for core in range(NCORES):
            for t in range(NTILES):
                for h in range(2):
                    n = len(per_core[core][m][t][h][0])
                    C_half[h, m, t] = max(C_half[h, m, t], (n + P - 1) // P)

    # chunk stream: list of (m, gi, half, t, c) in device order; per (m,gi,half)
    # the gather NIs (multiples of 128, each <= MAXIDX)
    stream = []
    gather_plan = []  # (m, gi, half, [NI, ...])
    for m in range(M):
        for gi, (t0, gt) in enumerate(GROUPS):
            for h in range(2):
                lanes = 0
                for t in range(t0, t0 + gt):
                    for c in range(C_half[h, m, t]):
                        stream.append((m, gi, h, t, c))
                    lanes += C_half[h, m, t] * P
                nis = [MAXIDX] * (lanes // MAXIDX)
                if lanes % MAXIDX:
                    nis.append(lanes % MAXIDX)
                gather_plan.append((m, gi, h, tuple(nis)))

    total_chunks = len(stream)
    total_lanes = total_chunks * P

    idx_all, drw_all = [], []
    for core in range(NCORES):
        idx_flat = np.zeros(total_lanes, np.int16)
        dr = np.zeros((P, total_chunks), np.float32)
        wg = np.zeros((P, total_chunks), np.float32)
        pos = 0
        for j, (m, gi, h, t, c) in enumerate(stream):
            s_t, d_t, w_t = per_core[core][m][t][h]
            a = c * P
            b = min(a + P, len(s_t))
            n = max(0, b - a)
            if n > 0:
                idx_flat[pos : pos + n] = s_t[a:b]
                dr[:n, j] = d_t[a:b]
                wg[:n, j] = w_t[a:b]
            pos += P
        # wrap: idx i -> partition i%16, col i//16; replicate to 8 groups
        wrapped = idx_flat.reshape(total_lanes // 16, 16).T  # [16, cols]
        idx_arr = np.tile(wrapped, (8, 1))                   # [128, cols]
        # drw layout: per (m, gi) block [dr chunks | w chunks] (both halves)
        blocks = []
        j = 0
        for m in range(M):
            for gi, (t0, gt) in enumerate(GROUPS):
                nch = sum(
                    C_half[h, m, t]
                    for h in range(2)
                    for t in range(t0, t0 + gt)
                )
                blocks.append(dr[:, j : j + nch])
                blocks.append(wg[:, j : j + nch])
                j += nch
        idx_all.append(np.ascontiguousarray(idx_arr))
        drw_all.append(np.ascontiguousarray(np.concatenate(blocks, axis=1)))

    meta = (
        tuple(map(tuple, C_half.reshape(2, -1).tolist())),
        tuple((m, gi, h, nis) for (m, gi, h, nis) in gather_plan),
    )
    return idx_all, drw_all, C_half, gather_plan, stream, meta


def _build_program(C_half, gather_plan, stream, meta):
    if meta in _PROGRAM_CACHE:
        return _PROGRAM_CACHE[meta]
    f32 = mybir.dt.float32
    bf16 = mybir.dt.bfloat16
    i16 = mybir.dt.int16
    NT = NTILES
    total_chunks = len(stream)
    total_lanes = total_chunks * P
    idx_cols = total_lanes // 16

    nc = bacc.Bacc(
        None, target_bir_lowering=False, num_devices=NCORES, num_swdge_queues=4
    )
    hlo_d = nc.dram_tensor("hlo", [HALF, D], bf16, kind="ExternalInput")
    hhi_d = nc.dram_tensor("hhi", [HALF, D], bf16, kind="ExternalInput")
    idx_d = nc.dram_tensor("idx", [P, idx_cols], i16, kind="ExternalInput")
    drw_d = nc.dram_tensor(
        "drw", [P, 2 * total_chunks], f32, kind="ExternalInput"
    )
    iota_d = nc.dram_tensor("iota", [P, P], bf16, kind="ExternalInput")
    w1_d = nc.dram_tensor("w1", [D, D], bf16, kind="ExternalInput")
    b1_d = nc.dram_tensor("b1", [D, 1], f32, kind="ExternalInput")
    z_d = nc.dram_tensor("z", [P, M * NT * P], bf16, kind="ExternalOutput")
    n_acc = M * NGRP
    acc_d = nc.dram_tensor("acc", [P, n_acc], f32, kind="ExternalOutput")

    # per-(m,gi): chunk-column base in drw blocks, idx-lane base, chunk list
    group_chunks = {}
    for j, (m, gi, h, t, c) in enumerate(stream):
        group_chunks.setdefault((m, gi), []).append((j, h, t, c))
    # first/last chunk of each (m, t) for psum start/stop flags
    first_chunk = {}
    last_chunk = {}
    for j, (m, gi, h, t, c) in enumerate(stream):
        if (m, t) not in first_chunk:
            first_chunk[(m, t)] = j
        last_chunk[(m, t)] = j

    with tile.TileContext(nc) as tc:
        with (
            tc.tile_pool(name="consts", bufs=1) as cpool,
            tc.tile_pool(name="zpool", bufs=1) as zpool,
            tc.tile_pool(name="accp", bufs=1) as apool,
            tc.tile_pool(name="meta", bufs=3) as mpool,
            tc.tile_pool(name="gather", bufs=4) as gpool,
            tc.tile_pool(name="sel", bufs=6) as spool,
            tc.tile_pool(name="psum_z", bufs=7, space="PSUM") as pz,
            tc.tile_pool(name="psum_y", bufs=1, space="PSUM") as py,
            tc.tile_pool(name="tanh", bufs=2) as tpool,
        ):
            iota_sb = cpool.tile([P, P], bf16)
            nc.sync.dma_start(out=iota_sb[:], in_=iota_d[:])
            w1_sb = cpool.tile([P, P], bf16)
            nc.sync.dma_start(out=w1_sb[:], in_=w1_d[:])
            b1_sb = cpool.tile([P, 1], f32)
            nc.sync.dma_start(out=b1_sb[:], in_=b1_d[:])

            z_sb = zpool.tile([P, M * NT * P], bf16)
            acc_sb = apool.tile([P, n_acc], f32)

            glist = [(m, gi) for m in range(M) for gi in range(NGRP)]

            # per-(m,gi) meta tiles: load idx + drw one group ahead
            def load_meta(u):
                m, gi = glist[u]
                chunks = group_chunks[(m, gi)]
                j0 = chunks[0][0]
                nch = len(chunks)
                lane0 = j0 * P
                nlanes = nch * P
                i_sb = mpool.tile([P, nlanes // 16], i16, tag=f"i{nlanes}")
                nc.sync.dma_start(
                    out=i_sb[:],
                    in_=idx_d[:, lane0 // 16 : (lane0 + nlanes) // 16],
                )
                dw_sb = mpool.tile([P, 2 * nch], f32, tag=f"d{nch}")
                nc.sync.dma_start(
                    out=dw_sb[:], in_=drw_d[:, 2 * j0 : 2 * (j0 + nch)]
                )
                return i_sb, dw_sb

            metas = {0: load_meta(0)}
            for u, (m, gi) in enumerate(glist):
                if u + 1 < len(glist):
                    metas[u + 1] = load_meta(u + 1)
                i_sb, dw_sb = metas.pop(u)
                chunks = group_chunks[(m, gi)]
                j0 = chunks[0][0]
                nch = len(chunks)
                t0, gt = GROUPS[gi]

                psums = {}
                for t in range(t0, t0 + gt):
                    psums[t] = pz.tile([P, P], f32, space="PSUM", tag="zt", name=f"psum_zt{t % 7}")

                # issue gathers for this group's two halves, then compute
                # chunk-by-chunk as data arrives
                plan = [
                    (h, nis)
                    for (pm, pgi, h, nis) in gather_plan
                    if pm == m and pgi == gi
                ]
                # lane offset (relative to group's start) per half
                lane_off = 0
                gather_tiles = []  # (g_sb, chunk_j_start, nchunks)
                for h, nis in plan:
                    src_d = hlo_d if h == 0 else hhi_d
                    for ni in nis:
                        g_sb = gpool.tile(
                            [P, ni // P, P], bf16, tag=f"g{ni}"
                        )
                        col0 = lane_off // 16
                        nc.gpsimd.dma_gather(
                            g_sb[:],
                            src_d[:],
                            i_sb[:, col0 : col0 + ni // 16],
                            ni,
                            ni,
                            D,
                        )
                        gather_tiles.append((g_sb, j0 + lane_off // P, ni // P))
                        lane_off += ni

                for g_sb, jstart, nb in gather_tiles:
                    for b in range(nb):
                        j = jstart + b
                        t = stream[j][3]
                        jj = j - j0  # chunk col within group's drw block
                        s_sb = spool.tile([P, P], bf16, tag="s")
                        nc.vector.tensor_scalar(
                            out=s_sb[:],
                            in0=iota_sb[:],
                            scalar1=dw_sb[:, jj : jj + 1],
                            scalar2=dw_sb[:, nch + jj : nch + jj + 1],
                            op0=mybir.AluOpType.is_equal,
                            op1=mybir.AluOpType.mult,
                        )
                        nc.tensor.matmul(
                            out=psums[t][:],
                            lhsT=g_sb[:, b, :],
                            rhs=s_sb[:],
                            start=(j == first_chunk[(m, t)]),
                            stop=(j == last_chunk[(m, t)]),
                        )

                for t in range(t0, t0 + gt):
                    nc.scalar.activation(
                        out=z_sb[:, (m * NT + t) * P : (m * NT + t + 1) * P],
                        in_=psums[t][:],
                        func=mybir.ActivationFunctionType.Copy,
                    )
                # fused score path: one <=512-col slab per group
                zc0 = (m * NT + t0) * P
                w = gt * P
                psum_y = py.tile([P, 512], f32, space="PSUM", tag="y")
                nc.tensor.matmul(
                    out=psum_y[:, :w],
                    lhsT=w1_sb[:],
                    rhs=z_sb[:, zc0 : zc0 + w],
                    start=True,
                    stop=True,
                )
                th_sb = tpool.tile([P, 512], bf16, tag="th")
                k = m * NGRP + gi
                nc.scalar.activation(
                    out=th_sb[:, :w],
                    in_=psum_y[:, :w],
                    func=mybir.ActivationFunctionType.Tanh,
                    bias=b1_sb[:, :1],
                    accum_out=acc_sb[:, k : k + 1],
                )

            nc.sync.dma_start(out=z_d[:], in_=z_sb[:])
            nc.sync.dma_start(out=acc_d[:], in_=acc_sb[:])
    nc.finalize()
    _PROGRAM_CACHE[meta] = nc
    return nc


def kernel(h, edges, W1, b1, W2):
    global LAST_RESULTS
    h = np.ascontiguousarray(np.asarray(h, dtype=np.float32))
    edges = np.asarray(edges)
    idx_all, drw_all, C_half, gather_plan, stream, meta = _preprocess(edges)
    nc = _build_program(C_half, gather_plan, stream, meta)

    h_bf = h.astype(BF16)
    hlo = np.ascontiguousarray(h_bf[:HALF])
    hhi = np.ascontiguousarray(h_bf[HALF:])
    iota = np.tile(np.arange(P, dtype=np.float32), (P, 1)).astype(BF16)
    w1 = np.asarray(W1, dtype=np.float32).astype(BF16)
    b1c = np.ascontiguousarray(np.asarray(b1, dtype=np.float32).reshape(D, 1))

    in_maps = []
    for core in range(NCORES):
        in_maps.append(
            {
                "hlo": hlo,
                "hhi": hhi,
                "idx": idx_all[core],
                "drw": drw_all[core],
                "iota": iota,
                "w1": w1,
                "b1": b1c,
            }
        )
    res = run_bass_kernel_spmd(
        nc, in_maps, core_ids=list(range(NCORES)), trace=TRACE
    )
    LAST_RESULTS = res

    # host epilogue: scores -> softmax -> beta-weighted combine + transpose
    n_acc = M * NGRP
    W2v = np.asarray(W2, dtype=np.float64).reshape(D)
    acc_m = np.zeros((M, D), np.float64)
    z_cores = []
    for core in range(NCORES):
        acc = np.asarray(res.results[core]["acc"], dtype=np.float64)
        for m in range(M):
            acc_m[m] += acc[:, m * NGRP : (m + 1) * NGRP].sum(axis=1)
        z_cores.append(np.asarray(res.results[core]["z"]).astype(np.float32))
    s = acc_m @ W2v  # [M]
    w = s / N
    e = np.exp(w - w.max())
    beta = (e / e.sum()).astype(np.float32)

    outs = []
    for core in range(NCORES):
        z3 = z_cores[core].reshape(P, M, NTILES * P)[:, :, :NSH]  # [d, m, dst]
        out_core = np.tensordot(beta, z3, axes=([0], [1]))  # [d, dst]
        outs.append(out_core.T)
    return np.ascontiguousarray(np.concatenate(outs, axis=0).astype(np.float32))


# revision 11
# speedup vs baseline: 1.7188x; 1.2793x over previous
"""HAN layer (3-metapath GraphConv + semantic attention) on 8 trn2 NeuronCores.

Strategy: shard destination nodes across the 8 cores (6250 rows each) and
partition each metapath's edge list by destination shard on the host.  Edges
are sorted by destination and cut into 128-edge chunks per 128-dst output
tile; chunk counts are padded to the cross-core maximum so one SPMD program
serves all cores.

The per-edge source-row gather uses the SWDGE `dma_gather` instruction.
Constraints found on this backend: int16 indices (so h is pre-cast to bf16
and split into two 25000-row halves, each tile's edges split into lo/hi
chunk runs), at most 1024 indices per instruction (descriptor-ring size),
and ~2.2us of Q7 descriptor-generation per 1024 rows, which is the kernel's
roofline.  Gathers are issued round-robin over the 4 SWDGE queues with 8
destination buffers so generation, transfer and compute fully overlap.

Because the GraphConv edge weight is separable (w_e = ns[src]*nd[dst]) it is
folded into the per-chunk selection matrix S[e, dst] = (dst_rel[e] == iota)
* w_e, built with one 4x-mode tensor_scalar (is_equal + mult) on DVE.  The
aggregation is one bf16 matmul per chunk accumulating z^T = g^T @ S
(d-major) in PSUM; each tile's psum accumulates its lo chunks then its hi
chunks and is copied to SBUF bf16 on the Scalar engine.

The semantic-attention score path runs fused per 4-tile slab: y^T = W1^T @
z^T (N<=512 matmul), tanh+bias on the Scalar engine with accum_out giving
per-partition column sums.  The tiny remainder (W2 dot, softmax over the 3
metapaths, beta-weighted combine of z, final transpose) runs on the host,
which removes the all-reduce and the device-side combine entirely.
"""

import numpy as np
import ml_dtypes

import concourse.bass as bass
import concourse.bacc as bacc
import concourse.mybir as mybir
import concourse.tile as tile
from concourse.bass_utils import run_bass_kernel_spmd

BF16 = ml_dtypes.bfloat16

P = 128
N = 50000
HALF = 25000
D = 128
M = 3
NCORES = 8
NSH = N // NCORES            # 6250 dst rows per core
NTILES = (NSH + P - 1) // P  # 49 output tiles (last has 106 real rows)
G = 4                        # dst tiles per score slab
GROUPS = [(t0, min(G, NTILES - t0)) for t0 in range(0, NTILES, G)]  # 13
NGRP = len(GROUPS)
MAXIDX = 1024                # dma_gather descriptor-ring limit
NQUEUES = 4
GATHER_AHEAD = 6             # gathers issued ahead of consumption

TRACE = False
LAST_RESULTS = None

_PROGRAM_CACHE = {}


def _preprocess(edges):
    """Host-side prep.

    Layouts (shared with the device program):
      idx  [P, total_lanes//16] int16: per (m, half) block; within a block,
           lanes ordered (t, c, p); 16-partition wrapped, replicated x8.
      drw  [P, 2*total_chunks] f32: per-m block [dr cols | w cols]; chunk
           columns ordered (t, lo chunks, hi chunks).
    """
    per_core = [[] for _ in range(NCORES)]
    for m in range(M):
        src = np.asarray(edges[m, 0])
        dst = np.asarray(edges[m, 1])
        out_deg = np.bincount(src, minlength=N).astype(np.float64)
        in_deg = np.bincount(dst, minlength=N).astype(np.float64)
        ns = 1.0 / np.sqrt(np.maximum(out_deg, 1.0))
        nd = 1.0 / np.sqrt(np.maximum(in_deg, 1.0))
        w_e = (ns[src] * nd[dst]).astype(np.float32)

        order = np.argsort(dst, kind="stable")
        src_s, dst_s, w_s = src[order], dst[order], w_e[order]
        shard_bounds = np.searchsorted(dst_s, NSH * np.arange(NCORES + 1))
        for core in range(NCORES):
            lo, hi = shard_bounds[core], shard_bounds[core + 1]
            sc, dc, wc = src_s[lo:hi], dst_s[lo:hi] - core * NSH, w_s[lo:hi]
            tile_id = dc >> 7
            tb = np.searchsorted(tile_id, np.arange(NTILES + 1))
            tiles = []
            for t in range(NTILES):
                a, b = tb[t], tb[t + 1]
                s_t, d_t, w_t = sc[a:b], dc[a:b] - t * P, wc[a:b]
                is_lo = s_t < HALF
                tiles.append(
                    (
                        (s_t[is_lo], d_t[is_lo], w_t[is_lo]),
                        (s_t[~is_lo] - HALF, d_t[~is_lo], w_t[~is_lo]),
                    )
                )
            per_core[core].append(tiles)

    # cross-core max chunk counts -> uniform SPMD program
    C_half = np.zeros((2, M, NTILES), np.int64)
    for m in range(M):
        for core in range(NCORES):
            for t in range(NTILES):
                for h in range(2):
                    n = len(per_core[core][m][t][h][0])
                    C_half[h, m, t] = max(C_half[h, m, t], (n + P - 1) // P)

    total_chunks = int(C_half.sum())
    total_lanes = total_chunks * P

    idx_all, drw_all = [], []
    for core in range(NCORES):
        idx_flat = np.zeros(total_lanes, np.int16)
        drcols = np.zeros((P, total_chunks), np.float32)
        wcols = np.zeros((P, total_chunks), np.float32)
        ipos = 0      # lane position (per (m, half) idx blocks)
        mcol0 = 0     # drw chunk-column base of metapath m
        for m in range(M):
            nch_m = int(C_half[:, m, :].sum())
            # drw chunk-column order within m: (t, lo chunks, hi chunks)
            chunk_col = {}
            jj = 0
            for t in range(NTILES):
                for h in range(2):
                    for c in range(C_half[h, m, t]):
                        chunk_col[(h, t, c)] = jj
                        jj += 1
            # idx order: half-major, then (t, c, p)
            for h in range(2):
                for t in range(NTILES):
                    s_t, d_t, w_t = per_core[core][m][t][h]
                    for c in range(C_half[h, m, t]):
                        a = c * P
                        b = min(a + P, len(s_t))
                        n = max(0, b - a)
                        j = mcol0 + chunk_col[(h, t, c)]
                        if n > 0:
                            idx_flat[ipos : ipos + n] = s_t[a:b]
                            drcols[:n, j] = d_t[a:b]
                            wcols[:n, j] = w_t[a:b]
                        ipos += P
            mcol0 += nch_m
        # drw: per-m block [dr cols | w cols]
        blocks = []
        mcol0 = 0
        for m in range(M):
            nch_m = int(C_half[:, m, :].sum())
            blocks.append(drcols[:, mcol0 : mcol0 + nch_m])
            blocks.append(wcols[:, mcol0 : mcol0 + nch_m])
            mcol0 += nch_m
        drw = np.ascontiguousarray(np.concatenate(blocks, axis=1))

        wrapped = idx_flat.reshape(total_lanes // 16, 16).T
        idx_all.append(np.ascontiguousarray(np.tile(wrapped, (8, 1))))
        drw_all.append(drw)

    meta = tuple(map(tuple, C_half.reshape(2, -1).tolist()))
    return idx_all, drw_all, C_half, meta


def _build_program(C_half, meta):
    if meta in _PROGRAM_CACHE:
        return _PROGRAM_CACHE[meta]
    f32 = mybir.dt.float32
    bf16 = mybir.dt.bfloat16
    i16 = mybir.dt.int16
    NT = NTILES
    total_chunks = int(C_half.sum())
    total_lanes = total_chunks * P
    idx_cols = total_lanes // 16

    nc = bacc.Bacc(
        None, target_bir_lowering=False, num_devices=NCORES, num_swdge_queues=4
    )
    hlo_d = nc.dram_tensor("hlo", [HALF, D], bf16, kind="ExternalInput")
    hhi_d = nc.dram_tensor("hhi", [HALF, D], bf16, kind="ExternalInput")
    idx_d = nc.dram_tensor("idx", [P, idx_cols], i16, kind="ExternalInput")
    drw_d = nc.dram_tensor(
        "drw", [P, 2 * total_chunks], f32, kind="ExternalInput"
    )
    iota_d = nc.dram_tensor("iota", [P, P], bf16, kind="ExternalInput")
    w1_d = nc.dram_tensor("w1", [D, D], bf16, kind="ExternalInput")
    b1_d = nc.dram_tensor("b1", [D, 1], f32, kind="ExternalInput")
    z_d = nc.dram_tensor("z", [P, M * NT * P], bf16, kind="ExternalOutput")
    n_acc = M * NGRP
    acc_d = nc.dram_tensor("acc", [P, n_acc], f32, kind="ExternalOutput")

    half_lanes = [
        [int(C_half[h, m, :].sum()) * P for h in range(2)] for m in range(M)
    ]
    idx_base = {}
    base = 0
    for m in range(M):
        for h in range(2):
            idx_base[(m, h)] = base
            base += half_lanes[m][h]
    gathers = {}  # (m, h) -> list of (lane0, ni)
    for m in range(M):
        for h in range(2):
            lanes = half_lanes[m][h]
            lst = []
            pos = 0
            while pos < lanes:
                ni = min(MAXIDX, lanes - pos)
                lst.append((pos, ni))
                pos += ni
            gathers[(m, h)] = lst

    with tile.TileContext(nc) as tc:
        with (
            tc.tile_pool(name="consts", bufs=1) as cpool,
            tc.tile_pool(name="zpool", bufs=1) as zpool,
            tc.tile_pool(name="accp", bufs=1) as apool,
            tc.tile_pool(name="meta", bufs=2) as mpool,
            tc.tile_pool(name="gather", bufs=8) as gpool,
            tc.tile_pool(name="sel", bufs=8) as spool,
            tc.tile_pool(name="psum_z", bufs=6, space="PSUM") as pz,
            tc.tile_pool(name="psum_y", bufs=2, space="PSUM") as py,
            tc.tile_pool(name="tanh", bufs=2) as tpool,
        ):
            iota_sb = cpool.tile([P, P], bf16)
            nc.sync.dma_start(out=iota_sb[:], in_=iota_d[:])
            w1_sb = cpool.tile([P, P], bf16)
            nc.sync.dma_start(out=w1_sb[:], in_=w1_d[:])
            b1_sb = cpool.tile([P, 1], f32)
            nc.sync.dma_start(out=b1_sb[:], in_=b1_d[:])

            z_sb = zpool.tile([P, M * NT * P], bf16)
            acc_sb = apool.tile([P, n_acc], f32)

            qcounter = [0]

            def load_meta(m):
                """idx tiles for both halves + drw tile for metapath m."""
                nch_m = int(C_half[:, m, :].sum())
                mcol0 = int(C_half[:, :m, :].sum()) if m else 0
                itiles = []
                for h in range(2):
                    lanes = half_lanes[m][h]
                    b0 = idx_base[(m, h)] // 16
                    i_sb = mpool.tile(
                        [P, lanes // 16], i16, tag=f"i{h}",
                        name=f"idx_h{h}",
                    )
                    nc.sync.dma_start(
                        out=i_sb[:], in_=idx_d[:, b0 : b0 + lanes // 16]
                    )
                    itiles.append(i_sb)
                dw_sb = mpool.tile([P, 2 * nch_m], f32, tag="d", name="drw_m")
                nc.sync.dma_start(
                    out=dw_sb[:],
                    in_=drw_d[:, 2 * mcol0 : 2 * (mcol0 + nch_m)],
                )
                return itiles, dw_sb

            metas = {0: load_meta(0)}
            for m in range(M):
                if m + 1 < M:
                    metas[m + 1] = load_meta(m + 1)
                i_sbs, dw_sb = metas.pop(m)
                nch_m = int(C_half[:, m, :].sum())

                glists = [gathers[(m, 0)], gathers[(m, 1)]]
                g_tiles = [{}, {}]
                issued = [0, 0]

                def issue(h, upto, i_sbs=i_sbs, glists=glists,
                          g_tiles=g_tiles, issued=issued, m=m):
                    src_d = hlo_d if h == 0 else hhi_d
                    while issued[h] <= upto and issued[h] < len(glists[h]):
                        lane0, ni = glists[h][issued[h]]
                        g_sb = gpool.tile(
                            [P, ni // P, P], bf16, tag=f"g{ni}",
                            name=f"gt{ni}",
                        )
                        nc.gpsimd.dma_gather(
                            g_sb[:],
                            src_d[:],
                            i_sbs[h][:, lane0 // 16 : (lane0 + ni) // 16],
                            ni,
                            ni,
                            D,
                            queue_num=qcounter[0] % NQUEUES,
                        )
                        qcounter[0] += 1
                        g_tiles[h][issued[h]] = g_sb
                        issued[h] += 1

                cum = [[0], [0]]
                for t in range(NT):
                    for h in range(2):
                        cum[h].append(cum[h][-1] + int(C_half[h, m, t]))

                issue(0, GATHER_AHEAD // 2)
                issue(1, GATHER_AHEAD // 2)

                jcol = 0
                for t in range(NT):
                    nlo = int(C_half[0, m, t])
                    nhi = int(C_half[1, m, t])
                    psum_zt = pz.tile(
                        [P, P], f32, space="PSUM", tag="zt",
                        name=f"psum_zt{t % 6}",
                    )
                    nchunks_t = nlo + nhi
                    ci = 0
                    for h, nh in ((0, nlo), (1, nhi)):
                        for c in range(nh):
                            lane = (cum[h][t] + c) * P
                            gidx = lane // MAXIDX
                            blk = (lane % MAXIDX) // P
                            issue(h, gidx + GATHER_AHEAD)
                            g_sb = g_tiles[h][gidx]
                            s_sb = spool.tile([P, P], bf16, tag="s")
                            nc.vector.tensor_scalar(
                                out=s_sb[:],
                                in0=iota_sb[:],
                                scalar1=dw_sb[:, jcol : jcol + 1],
                                scalar2=dw_sb[
                                    :, nch_m + jcol : nch_m + jcol + 1
                                ],
                                op0=mybir.AluOpType.is_equal,
                                op1=mybir.AluOpType.mult,
                            )
                            nc.tensor.matmul(
                                out=psum_zt[:],
                                lhsT=g_sb[:, blk, :],
                                rhs=s_sb[:],
                                start=(ci == 0),
                                stop=(ci == nchunks_t - 1),
                            )
                            jcol += 1
                            ci += 1
                    nc.scalar.activation(
                        out=z_sb[:, (m * NT + t) * P : (m * NT + t + 1) * P],
                        in_=psum_zt[:],
                        func=mybir.ActivationFunctionType.Copy,
                    )
                    gi = t // G
                    t0, gt = GROUPS[gi]
                    if t == t0 + gt - 1:
                        zc0 = (m * NT + t0) * P
                        w = gt * P
                        psum_y = py.tile(
                            [P, 512], f32, space="PSUM", tag="y", name="psum_y"
                        )
                        nc.tensor.matmul(
                            out=psum_y[:, :w],
                            lhsT=w1_sb[:],
                            rhs=z_sb[:, zc0 : zc0 + w],
                            start=True,
                            stop=True,
                        )
                        th_sb = tpool.tile([P, 512], bf16, tag="th")
                        k = m * NGRP + gi
                        nc.scalar.activation(
                            out=th_sb[:, :w],
                            in_=psum_y[:, :w],
                            func=mybir.ActivationFunctionType.Tanh,
                            bias=b1_sb[:, :1],
                            accum_out=acc_sb[:, k : k + 1],
                        )

            nc.sync.dma_start(out=z_d[:], in_=z_sb[:])
            nc.sync.dma_start(out=acc_d[:], in_=acc_sb[:])
    nc.finalize()
    _PROGRAM_CACHE[meta] = nc
    return nc


def kernel(h, edges, W1, b1, W2):
    global LAST_RESULTS
    h = np.ascontiguousarray(np.asarray(h, dtype=np.float32))
    edges = np.asarray(edges)
    idx_all, drw_all, C_half, meta = _preprocess(edges)
    nc = _build_program(C_half, meta)

    h_bf = h.astype(BF16)
    hlo = np.ascontiguousarray(h_bf[:HALF])
    hhi = np.ascontiguousarray(h_bf[HALF:])
    iota = np.tile(np.arange(P, dtype=np.float32), (P, 1)).astype(BF16)
    w1 = np.asarray(W1, dtype=np.float32).astype(BF16)
    b1c = np.ascontiguousarray(np.asarray(b1, dtype=np.float32).reshape(D, 1))

    in_maps = []
    for core in range(NCORES):
        in_maps.append(
            {
                "hlo": hlo,
                "hhi": hhi,
                "idx": idx_all[core],
                "drw": drw_all[core],
                "iota": iota,
                "w1": w1,
                "b1": b1c,
            }
        )
    res = run_bass_kernel_spmd(
        nc, in_maps, core_ids=list(range(NCORES)), trace=TRACE
    )
    LAST_RESULTS = res

    # host epilogue: scores -> softmax -> beta-weighted combine + transpose
    W2v = np.asarray(W2, dtype=np.float64).reshape(D)
    acc_m = np.zeros((M, D), np.float64)
    z_cores = []
    for core in range(NCORES):
        acc = np.asarray(res.results[core]["acc"], dtype=np.float64)
        for m in range(M):
            acc_m[m] += acc[:, m * NGRP : (m + 1) * NGRP].sum(axis=1)
        z_cores.append(np.asarray(res.results[core]["z"]).astype(np.float32))
    s = acc_m @ W2v
    w = s / N
    e = np.exp(w - w.max())
    beta = (e / e.sum()).astype(np.float32)

    outs = []
    for core in range(NCORES):
        z3 = z_cores[core].reshape(P, M, NTILES * P)[:, :, :NSH]  # [d, m, dst]
        out_core = np.tensordot(beta, z3, axes=([0], [1]))  # [d, dst]
        outs.append(out_core.T)
    return np.ascontiguousarray(np.concatenate(outs, axis=0).astype(np.float32))
